# revision 14
# baseline (speedup 1.0000x reference)
"""Deformable-conv Trainium2 kernel (8-core SPMD, bass/Tile).

Per-core shard: core = (b, half): b = core//2, h0 = 60*(core%2).
Device pipeline per 2-row batch (r0 = 2*it):
  1. offset conv on PE: 50 tap-matmuls -> PSUM [120w, 200ch] (2 rows x 100)
     ch layout per row: [0:50) = off_y slots (g*25+k), [50:100) = off_x
  2. positions/idx/weights on DVE in [w-part, free] layout
  3. idx marshalling: SBUF->DRAM->SBUF into the SWDGE wrapped-int16 layout
  4. ONE dma_gather: 12800 tokens x 256B from bf16 windowed volume
     (token t = (r*50+sl)*128 + w lands at partition w, free slot r*50+sl)
  5. bilinear combine on DVE (bf16, packed-pair weights, 2x mode)
  6. PE transposes -> einsum chunk matmuls (bf16) -> PSUM [120,32]x4 -> bias -> DRAM
"""
"""Patch TileContext tail-drain: this walrus build rejects >2 sync waits per instruction."""
import sys
for _p in ("/opt/trn_rl_repo", "/root/.axon_site/_ro/trn_rl_repo"):
    import os as _os
    if _os.path.isdir(_p) and _p not in sys.path:
        sys.path.insert(0, _p)
import bass_rust
import concourse.tile as tile
from concourse.vector_clock import ScopedClock

_MAX_WAITS = 1

def _patched_drain_and_barrier(self, tick_clock, wait_clock):
    nc = self.nc
    drain_inst = nc.sync.drain()
    wait_clock.add_sem_waits(drain_inst.ins, ScopedClock({None: tick_clock.global_clock}))
    raw = drain_inst.ins
    si = raw.sync_info
    waits = list(si.on_wait or []) if si is not None else []
    if len(waits) > _MAX_WAITS:
        si.on_wait = waits[:_MAX_WAITS]
        rest = waits[_MAX_WAITS:]
        for i in range(0, len(rest), _MAX_WAITS):
            extra = nc.sync.drain()
            eraw = extra.ins
            chunk = rest[i:i + _MAX_WAITS]
            if eraw.sync_info is None:
                eraw.sync_info = bass_rust.SyncInfo(on_wait=chunk, on_update=[])
            else:
                eraw.sync_info.on_wait = chunk

    nc.all_engine_barrier()
    assert self.sems is not None
    popped = nc._tile_sem_poison_stack.pop()
    assert popped is self._sem_poison
    nc.clear_and_free_semaphores(list(self.sems.allocated().values()))
    nc.all_engine_barrier()

tile.TileContext._drain_and_barrier = _patched_drain_and_barrier


def split_multi_waits(nc, max_waits=1):
    """Walrus in this build rejects >1 sync wait per instruction: hoist extras
    onto NOPs inserted just before, on the same engine."""
    import concourse.mybir as mybir
    for f in nc.m.functions:
        for bb in f.blocks:
            insts = bb.instructions
            i = 0
            while i < len(insts):
                inst = insts[i]
                si = inst.sync_info
                if si is not None and si.on_wait and len(si.on_wait) > max_waits:
                    waits = list(si.on_wait)
                    si.on_wait = waits[-max_waits:]
                    extra = waits[:-max_waits]
                    nops = []
                    for j in range(0, len(extra), max_waits):
                        n = mybir.InstNoOp(name=f"{inst.name}-w{j}", ins=[], outs=[])
                        n.engine = inst.engine
                        n.sync_info = bass_rust.SyncInfo(
                            on_wait=extra[j:j + max_waits], on_update=[])
                        nops.append(n)
                    for k, n in enumerate(nops):
                        insts.insert(i + k, n)
                        try:
                            nc.register_instruction(n, overwrite=True)
                        except Exception:
                            pass
                    i += len(nops)
                i += 1


# Enable DynamicDMA lowering in walrus (indirect/offset-table DMAs).
import concourse.bass_utils as _bu
_orig_gwa = _bu.get_walrus_args

def _gwa_dyn(*a, **k):
    return _orig_gwa(*a, **k) + [
        "--dge-levels=io,spill_reload,scalar_dynamic_offset,vector_dynamic_offsets",
    ]

if _bu.get_walrus_args is not _gwa_dyn:
    _bu.get_walrus_args = _gwa_dyn


import numpy as np
import concourse.bass as bass
import concourse.bacc as bacc
import concourse.mybir as mybir

F32 = mybir.dt.float32
BF16 = mybir.dt.bfloat16
I32 = mybir.dt.int32
I16 = mybir.dt.int16
Alu = mybir.AluOpType

H = 128; W = 128; C = 32
K = 25; G = 2; Fh = 5; Fw = 5; OW = 120
NCH = 100          # offset channels per row (y-block 50 | x-block 50)
NS = 50            # (g,k) slots
HPC = 60           # output rows per core
RB = 2             # rows per batch
NIDX = RB * NS * 128   # gather tokens per batch (incl pad partitions)
VQROWS = H * W + 136   # windowed gather-source rows


def host_prep(volume, w_off, b_off, w_dcn, b_dcn, n_cores=8, hpc=HPC):
    """Per-core input maps. Pure layout permutation / replication marshalling."""
    conv_rows = hpc + 8
    # permuted w_off: ch' = axis*50 + g*25 + k  <-  ch = k*4 + axis*2 + g
    chp = np.empty(NCH, np.int64)
    for axis in range(2):
        for g in range(G):
            for k in range(K):
                chp[axis * 50 + g * 25 + k] = k * (2 * G) + axis * G + g
    w_offT = np.ascontiguousarray(
        w_off.reshape(Fh * Fw, C, NCH)[:, :, chp]).astype(np.float32)  # [25, 32, 100]

    kys = np.arange(-4, 5, 2, np.float32)
    kxs = np.arange(-4, 5, 2, np.float32)
    kus, kvs = np.meshgrid(kxs, kys)
    kdy = kvs.reshape(-1); kdx = kus.reshape(-1)          # tap k = ky*5 + kx
    posk = np.empty(NCH, np.float32)
    for g in range(G):
        posk[g * 25:(g + 1) * 25] = kdy + 4.0
        posk[50 + g * 25:50 + (g + 1) * 25] = kdx + 4.0
    posk = posk + b_off[chp].astype(np.float32)
    posadd2 = np.tile(np.tile(posk, RB)[None, :], (128, 1)).astype(np.float32)

    iota_w = np.arange(128, dtype=np.float32)[:, None].copy()
    ident = np.eye(128, dtype=np.float32)

    wr = w_dcn.reshape(K, C, G, 32)
    wdT = np.zeros((128, 2 * 7, 32), np.float32)
    for g in range(G):
        for j in range(7):
            for i, k in enumerate(range(4 * j, min(4 * j + 4, K))):
                wdT[32 * i:32 * (i + 1), g * 7 + j, :] = wr[k, :, g, :]
    b_dcn_t = np.tile(b_dcn[None, :], (128, RB)).astype(np.float32)

    in_maps = []
    for core in range(n_cores):
        b = core // 2
        h0 = HPC * (core % 2)
        vol_full = np.ascontiguousarray(volume[b].reshape(H * W, C)).astype(np.float32)
        vol_conv = np.ascontiguousarray(
            volume[b, h0:h0 + conv_rows].reshape(conv_rows * W, C)).astype(np.float32)
        h0v = np.full((128, 1), float(h0), np.float32)
        in_maps.append({
            "vol_full": vol_full, "vol_conv": vol_conv,
            "w_offT": w_offT, "posadd2": posadd2,
            "iota_w": iota_w, "ident_f": ident,
            "wdT": wdT, "b_dcn_t": b_dcn_t,
            "h0v": h0v,
        })
    return in_maps


def build_nc(hpc=HPC):
    conv_rows = hpc + 8
    nbatch = hpc // RB
    nc = bacc.Bacc("TRN2", target_bir_lowering=False, debug=False,
                   num_swdge_queues=4)
    vol_full = nc.dram_tensor("vol_full", [H * W, C], F32, kind="ExternalInput")
    vol_conv = nc.dram_tensor("vol_conv", [conv_rows * W, C], F32, kind="ExternalInput")
    w_offT = nc.dram_tensor("w_offT", [K, C, NCH], F32, kind="ExternalInput")
    posadd2 = nc.dram_tensor("posadd2", [128, RB * NCH], F32, kind="ExternalInput")
    iota_w = nc.dram_tensor("iota_w", [128, 1], F32, kind="ExternalInput")
    ident_f = nc.dram_tensor("ident_f", [128, 128], F32, kind="ExternalInput")
    wdT = nc.dram_tensor("wdT", [128, 14, 32], F32, kind="ExternalInput")
    b_dcn_t = nc.dram_tensor("b_dcn_t", [128, RB * 64], F32, kind="ExternalInput")
    h0v = nc.dram_tensor("h0v", [128, 1], F32, kind="ExternalInput")
    out = nc.dram_tensor("out", [hpc, OW, 64], F32, kind="ExternalOutput")
    # gather source: full 2x2 patch per pixel [v00|v01|v10|v11] bf16, 256B rows
    volq = nc.dram_tensor("volq", [VQROWS, 4 * C], BF16)
    # idx marshalling scratch: per-batch [128, RB*NS] int16 regions
    idxscr = nc.dram_tensor("idxscr", [nbatch * 128 * RB * NS], I16)

    with tile.TileContext(nc) as tc:
        with (
            tc.tile_pool(name="stage", bufs=2) as stp,
            tc.tile_pool(name="res", bufs=1) as resp,
            tc.tile_pool(name="psA", bufs=2, space="PSUM") as psA,   # conv out + staging transposes
            tc.tile_pool(name="psB", bufs=3, space="PSUM") as psB,   # einsum transposes
            tc.tile_pool(name="psC", bufs=2, space="PSUM") as psC,   # einsum out
            tc.tile_pool(name="work", bufs=2) as wkp,
            tc.tile_pool(name="gtp", bufs=3) as gtp,
        ):
            # ---------- resident tiles ----------
            volT = resp.tile([C, conv_rows * W], BF16)   # [c, (y,x)] conv source
            w_offs = resp.tile([C, K * NCH], BF16)
            wds = resp.tile([128, 14 * 32], BF16)
            pos_c = resp.tile([128, RB * NCH], F32)
            iw = resp.tile([128, 1], F32)
            h0t = resp.tile([128, 1], F32)
            idn = resp.tile([128, 128], BF16)
            bdc = resp.tile([128, RB * 64], F32)

            # ---------- staging ----------
            nc.sync.dma_start(iw[:], iota_w[:])
            nc.sync.dma_start(h0t[:], h0v[:])
            nc.sync.dma_start(bdc[:], b_dcn_t[:])
            nc.sync.dma_start(pos_c[:], posadd2[:])
            idnf = stp.tile([128, 128], F32, tag="idnf")
            nc.sync.dma_start(idnf[:], ident_f[:])
            nc.vector.tensor_copy(idn[:], idnf[:])
            wof = stp.tile([C, K * NCH], F32, tag="wof")
            # w_offT dram [K, C, NCH] -> SBUF [C, (k, ch)]
            nc.sync.dma_start(wof[:], bass.AP(w_offT[:].tensor, 0,
                                              [[NCH, C], [C * NCH, K], [1, NCH]]))
            nc.vector.tensor_copy(w_offs[:], wof[:])
            wdsf = stp.tile([128, 14 * 32], F32, tag="wdsf")
            nc.sync.dma_start(wdsf[:], wdT[:].rearrange("p a b -> p (a b)"))
            nc.vector.tensor_copy(wds[:], wdsf[:])

            # volq[r + 132 - dy*128 - dx, (dy*2+dx)*32 : +32] = vol[r]  (bf16)
            for j in range(16):
                ch = stp.tile([128, 8 * 32], F32, tag="stg_in")
                nc.sync.dma_start(ch[:], bass.AP(vol_full[:].tensor, j * 128 * 8 * 32,
                                                 [[8 * 32, 128], [1, 8 * 32]]))
                chv = stp.tile([128, 8 * 32], BF16, tag="stg_bf")
                nc.vector.tensor_copy(chv[:], ch[:])
                for sft in range(4):
                    dy, dx = sft >> 1, sft & 1
                    nc.sync.dma_start(
                        bass.AP(volq[:].tensor,
                                (j * 1024 + 132 - dy * 128 - dx) * 128 + sft * 32,
                                [[8 * 128, 128], [128, 8], [1, 32]]),
                        chv[:].rearrange("p (r c) -> p r c", c=32))

            # volT: load vol_conv as [x-part, (y, c)], cast, then per-y PE-transpose
            vcx = resp.tile([W, conv_rows * C], BF16)
            vcf = stp.tile([W, conv_rows * C], F32, tag="vcf")
            nc.sync.dma_start(vcf[:], bass.AP(vol_conv[:].tensor, 0,
                                              [[C, W], [W * C, conv_rows], [1, C]]))
            nc.vector.tensor_copy(vcx[:], vcf[:])
            for y4 in range(0, conv_rows, 4):
                pt = psA.tile([C, 4 * W], BF16, space="PSUM", tag="conv")
                for i in range(4):
                    y = y4 + i
                    nc.tensor.transpose(out=pt[:, i * W:(i + 1) * W],
                                        in_=vcx[:, y * C:(y + 1) * C], identity=idn[:])
                nc.scalar.copy(volT[:, y4 * W:(y4 + 4) * W], pt[:])

            # ---------- per 2-row batch ----------
            _gq = [0]
            for it in range(nbatch):
                r0 = RB * it
                # 1. offset conv for both rows into one PSUM tile [120, 200]
                cps = psA.tile([OW, RB * NCH], F32, space="PSUM", tag="conv")
                for r in range(RB):
                    for ky in range(Fh):
                        for kx in range(Fw):
                            k = ky * 5 + kx
                            o = (r0 + r + 2 * ky) * W + 2 * kx
                            nc.tensor.matmul(out=cps[:, r * NCH:(r + 1) * NCH],
                                             lhsT=volT[:, o:o + OW],
                                             rhs=w_offs[:, k * NCH:(k + 1) * NCH],
                                             start=(k == 0), stop=(k == K - 1))
                # 2. positions po = cps + pos  (+h0+row on y-halves, +w on x-halves)
                po = wkp.tile([OW, RB * NCH], F32, tag="po")
                nc.vector.tensor_tensor(out=po[:], in0=cps[:], in1=pos_c[0:OW, :], op=Alu.add)
                for r in range(RB):
                    nc.vector.tensor_scalar(out=po[:, r * NCH:r * NCH + NS],
                                            in0=po[:, r * NCH:r * NCH + NS],
                                            scalar1=h0t[0:OW, :], scalar2=float(r0 + r),
                                            op0=Alu.add, op1=Alu.add)

                def halves(t, off):
                    a = t[:]
                    return bass.AP(a.tensor, a.offset + off,
                                   [a.ap[0], [NCH, RB], [1, NS]])

                nc.vector.tensor_scalar(out=halves(po, NS), in0=halves(po, NS),
                                        scalar1=iw[0:OW, :], scalar2=None, op0=Alu.add)
                nc.vector.tensor_scalar(out=po[:], in0=po[:], scalar1=0.0, scalar2=127.0,
                                        op0=Alu.max, op1=Alu.min)
                # 3. base = clip(floor(po), 0, 126) via round(po-0.5) fp32 trick
                base = wkp.tile([OW, RB * NCH], F32, tag="base")
                nc.vector.tensor_scalar(out=base[:], in0=po[:], scalar1=-0.5,
                                        scalar2=float(3 * 2**22), op0=Alu.add, op1=Alu.add)
                nc.vector.tensor_scalar(out=base[:], in0=base[:], scalar1=-float(3 * 2**22),
                                        scalar2=126.0, op0=Alu.add, op1=Alu.min)
                wgt = wkp.tile([OW, RB * NCH], F32, tag="wgt")
                nc.vector.tensor_tensor(out=wgt[:], in0=po[:], in1=base[:], op=Alu.subtract)
                # 4. gather indices: idx[w, r*NS+sl] = by*128 + bx + 132
                idxf = wkp.tile([128, RB * NS], F32, tag="idxf")
                nc.vector.memset(idxf[96:128, :], 132.0)

                def iview(t):
                    a = t[:]
                    return bass.AP(a.tensor, a.offset, [[a.ap[0][0], OW], [NS, RB], [1, NS]])

                nc.vector.tensor_scalar(out=iview(idxf), in0=halves(base, 0), scalar1=128.0,
                                        scalar2=132.0, op0=Alu.mult, op1=Alu.add)
                nc.vector.tensor_tensor(out=iview(idxf), in0=iview(idxf),
                                        in1=halves(base, NS), op=Alu.add)
                idx16 = wkp.tile([128, RB * NS], I16, tag="idx16")
                nc.vector.tensor_copy(idx16[:], idxf[:])
                # 5. marshal to wrapped SWDGE layout via DRAM roundtrip
                sbase = it * 128 * RB * NS
                nc.sync.dma_start(
                    bass.AP(idxscr[:].tensor, sbase, [[RB * NS, 128], [1, RB * NS]]),
                    idx16[:])
                wrapped = wkp.tile([128, NIDX // 16], I16, tag="wrapped")
                # wrapped[16k+p16, rsl*8+w16] = idx16[w16*16+p16, rsl], replicated over k
                for k8 in range(8):
                    wv = wrapped[16 * k8:16 * (k8 + 1), :]
                    nc.sync.dma_start(
                        bass.AP(wv.tensor, wv.offset, [wv.ap[0], [8, RB * NS], [1, 8]]),
                        bass.AP(idxscr[:].tensor, sbase,
                                [[RB * NS, 16], [1, RB * NS], [16 * RB * NS, 8]]))
                # 6. bilinear weights, bf16, x2-duplicated pairs for packed 2x DVE mode
                #    wqb2[w, r*400 + q*100 + sl*2 + d] = wq_q[w, r, sl]
                omw = wkp.tile([OW, RB * NCH], F32, tag="omw")
                nc.vector.tensor_scalar(out=omw[:], in0=wgt[:], scalar1=-1.0, scalar2=1.0,
                                        op0=Alu.mult, op1=Alu.add)
                wq = wkp.tile([OW, RB * 4 * NS], BF16, tag="wq")

                def qview(t, q):
                    a = t[:]
                    return bass.AP(a.tensor, a.offset + q * NS,
                                   [a.ap[0], [4 * NS, RB], [1, NS]])

                nc.vector.tensor_tensor(out=qview(wq, 0), in0=halves(omw, 0),
                                        in1=halves(omw, NS), op=Alu.mult)
                nc.vector.tensor_tensor(out=qview(wq, 1), in0=halves(omw, 0),
                                        in1=halves(wgt, NS), op=Alu.mult)
                nc.vector.tensor_tensor(out=qview(wq, 2), in0=halves(wgt, 0),
                                        in1=halves(omw, NS), op=Alu.mult)
                nc.vector.tensor_tensor(out=qview(wq, 3), in0=halves(wgt, 0),
                                        in1=halves(wgt, NS), op=Alu.mult)
                wqb2 = wkp.tile([OW, RB * 4 * NS, 2], BF16, tag="wqb2")
                nc.vector.tensor_copy(
                    wqb2[:], wq[:].unsqueeze(2).broadcast_to([OW, RB * 4 * NS, 2]))
                # 7. gather: 12800 tokens x 256B in 1024-token SWDGE chunks
                #    (per-gather descriptor-ring capacity caps num_idxs ~1024);
                #    rotate the 4 SWDGE queues so transfers overlap desc-gen
                gt = gtp.tile([128, RB * NS, 4 * C], BF16, tag="gt")
                for t0 in range(0, NIDX, 1024):
                    n = min(1024, NIDX - t0)
                    nc.gpsimd.dma_gather(gt[:, t0 // 128:(t0 + n) // 128, :], volq[:],
                                         wrapped[:, t0 // 16:(t0 + n) // 16],
                                         n, n, 4 * C, queue_num=_gq[0] % 4)
                    _gq[0] += 1
                # 8. combine on DVE (bf16 2x): T[w, r,(g,k),c] = bilinear of 4 corners
                T = wkp.tile([OW, RB * NS * C], BF16, tag="T")
                tm0 = wkp.tile([OW, NS * C], BF16, tag="tm0")
                tm1 = wkp.tile([OW, NS * C], BF16, tag="tm1")

                def gv(r, pp):
                    a = gt[:]
                    return bass.AP(a.tensor, a.offset + (r * NS) * 4 * C + pp * C,
                                   [[a.ap[0][0], OW], [4 * C, NS], [1, C]])

                def sv(t, r=0):
                    a = t[:]
                    return bass.AP(a.tensor, a.offset + r * NS * C,
                                   [[a.ap[0][0], OW], [C, NS], [1, C]])

                def wb(r, q):
                    a = wqb2[:]
                    return bass.AP(a.tensor, a.offset + (r * 4 * NS + q * NS) * 2,
                                   [a.ap[0], [2, NS], [0, C // 2], [1, 2]])

                for r in range(RB):
                    nc.vector.tensor_tensor(out=sv(tm0), in0=gv(r, 0), in1=wb(r, 0), op=Alu.mult)
                    nc.vector.tensor_tensor(out=sv(tm1), in0=gv(r, 1), in1=wb(r, 1), op=Alu.mult)
                    nc.vector.tensor_tensor(out=sv(T, r), in0=sv(tm0), in1=sv(tm1), op=Alu.add)
                    nc.vector.tensor_tensor(out=sv(tm0), in0=gv(r, 2), in1=wb(r, 2), op=Alu.mult)
                    nc.vector.tensor_tensor(out=sv(tm1), in0=gv(r, 3), in1=wb(r, 3), op=Alu.mult)
                    nc.vector.tensor_tensor(out=sv(tm0), in0=sv(tm0), in1=sv(tm1), op=Alu.add)
                    nc.vector.tensor_tensor(out=sv(T, r), in0=sv(T, r), in1=sv(tm0), op=Alu.add)
                # 9. einsum: per (r,g): transpose 7 chunks (bf16) into one PSUM
                #    tile, ACT-copy to SBUF, then 7 matmuls accumulating [120,32]
                opsO = psC.tile([OW, RB * G * 32], F32, space="PSUM", tag="eo")
                for r in range(RB):
                    for g in range(G):
                        rg = r * G + g
                        tps = psB.tile([128, 7 * OW], BF16, space="PSUM", tag="tsp")
                        for j in range(7):
                            wd = 128 if j < 6 else 32
                            c0 = r * 1600 + g * 800 + j * 128
                            nc.tensor.matmul(out=tps[0:wd, j * OW:(j + 1) * OW],
                                             lhsT=T[:, c0:c0 + wd],
                                             rhs=idn[0:OW, 0:OW], is_transpose=True,
                                             start=True, stop=True)
                        tss = wkp.tile([128, 7 * OW], BF16, tag="tss")
                        nc.scalar.copy(tss[:, 0:6 * OW], tps[:, 0:6 * OW])
                        nc.scalar.copy(tss[0:32, 6 * OW:], tps[0:32, 6 * OW:])
                        for j in range(7):
                            wd = 128 if j < 6 else 32
                            nc.tensor.matmul(
                                out=opsO[:, rg * 32:(rg + 1) * 32],
                                lhsT=tss[0:wd, j * OW:(j + 1) * OW],
                                rhs=wds[0:wd, (g * 7 + j) * 32:(g * 7 + j + 1) * 32],
                                start=(j == 0), stop=(j == 6))
                # 10. bias + out (both rows in one DMA; bdc is bias tiled x2)
                ot = wkp.tile([OW, RB * 64], F32, tag="ot")
                nc.vector.tensor_tensor(out=ot[:], in0=opsO[:],
                                        in1=bdc[0:OW, :], op=Alu.add)
                nc.sync.dma_start(
                    bass.AP(out[:].tensor, r0 * OW * 64,
                            [[64, OW], [OW * 64, RB], [1, 64]]),
                    ot[:])
    nc.compile()
    split_multi_waits(nc)
    return nc


_NC_CACHE = {}


def kernel(volume, w_off, b_off, w_dcn, b_dcn):
    """Deformable conv on 8 trn2 cores: full inputs in, full output out."""
    import numpy as _np
    from concourse.bass_utils import run_bass_kernel_spmd
    volume = _np.asarray(volume, _np.float32)
    w_off = _np.asarray(w_off, _np.float32)
    b_off = _np.asarray(b_off, _np.float32)
    w_dcn = _np.asarray(w_dcn, _np.float32)
    b_dcn = _np.asarray(b_dcn, _np.float32)
    in_maps = host_prep(volume, w_off, b_off, w_dcn, b_dcn)
    if "nc" not in _NC_CACHE:
        _NC_CACHE["nc"] = build_nc(hpc=HPC)
    nc = _NC_CACHE["nc"]
    res = run_bass_kernel_spmd(nc, in_maps, list(range(8)))
    out = _np.empty((4, 120, 120, 64), _np.float32)
    for core in range(8):
        b = core // 2
        h0 = HPC * (core % 2)
        out[b, h0:h0 + HPC] = res.results[core]["out"]
    return out
